# revision 18
# baseline (speedup 1.0000x reference)
"""GAT BasicAttentionBlock kernel for 8x Trainium2 NeuronCores.

Strategy (output-shard): each core owns 1250 of the 10000 selected output
rows (index0).  Only nodes reachable from those rows matter: ~1.2k unique
target nodes and ~16k unique source nodes per core (~5.5x less edge work
than the full graph).  Per core:

  phase A  gather x rows of needed nodes (host "halo"), compute
           h = relu(x@w1.T + b) feature-major on PE, then per 128-node
           subtile proj = h@w_proj.T and s_src = h@B_src node-major into a
           512-byte/row HBM table  [proj bf16 128 | s_src f32 8 | pad].
  phase B  per 128-target window: dma_gather the table rows of each edge's
           source (512B granules), build one-hot edge->target matrices on
           DVE (is_equal vs iota), expand per-target s_trg to edges with a
           tiny PE matmul, scores = leakyrelu(s_src+s_trg) exactly on DVE,
           exp on ACT, weighted = exp * proj, then segment-sum via one-hot
           matmuls accumulated in PSUM:  out[t] = [sum exp*proj | sum exp].
           out = att/den + skip (skip/s_trg matmuls reuse the targets' h
           kept resident in SBUF), ELU, store.
  final    dma_gather the 1250 output rows from the 1280-target table.

No collectives: cores are fully independent.  The softmax max-subtraction
in the reference cancels in the att = exp/sum(exp) ratio and is dropped
(scores are O(1) here, exp cannot overflow).
"""

import os
import sys

for _p in ("/opt/trn_rl_repo",):
    if os.path.isdir(_p) and _p not in sys.path:
        sys.path.insert(0, _p)

import numpy as np
import ml_dtypes

# problem constants (hardcoded per contract)
N = 50000
E = 800000
K = 10000
IN = 256
H = 128
NH = 8
HD = 16
OC = NH * HD  # 128
CORES = 8
KC = K // CORES          # 1250 output rows per core
P = 128
W = 10                   # target windows of 128 -> 1280 target slots
TP = W * P               # padded target count per core
EPS = 1e-16

BF16 = ml_dtypes.bfloat16


# ----------------------------------------------------------------------------
# host-side sharding / planning
# ----------------------------------------------------------------------------

def _wrap16(vals, reps=8):
    """int16 index layout for dma_gather: idx i at [i%16, i//16], the 16-row
    block replicated `reps` times down the partition axis."""
    L = vals.shape[0]
    assert L % 16 == 0
    w = vals.reshape(L // 16, 16).T.astype(np.int16)
    return np.tile(w, (reps, 1))


def plan(x, adj0, index0):
    src_all = np.asarray(adj0[0], dtype=np.int64)
    trg_all = np.asarray(adj0[1], dtype=np.int64)
    idx0 = np.asarray(index0, dtype=np.int64)
    x = np.asarray(x, dtype=np.float32)

    pre = []
    ec_req = 1
    npad_req = 512
    for c in range(CORES):
        ks = idx0[c * KC:(c + 1) * KC]
        tgt_u, inv_k = np.unique(ks, return_inverse=True)
        U_t = len(tgt_u)
        assert U_t <= TP
        lut = np.full(N, -1, np.int64)
        lut[tgt_u] = np.arange(U_t)
        tloc_all = lut[trg_all]
        sel = np.nonzero(tloc_all >= 0)[0]
        e_src = src_all[sel]
        e_tloc = tloc_all[sel]
        order = np.argsort(e_tloc, kind="stable")
        e_src = e_src[order]
        e_tloc = e_tloc[order]
        e_win = e_tloc >> 7
        cnt = np.bincount(e_win, minlength=W)
        ec_req = max(ec_req, int(np.ceil(cnt.max() / P)))

        extra = np.setdiff1d(np.unique(e_src), tgt_u)
        nodes = np.concatenate([tgt_u, extra])
        npad_req = max(npad_req, len(nodes))
        pre.append((tgt_u, inv_k, e_src, e_tloc, e_win, cnt, nodes))

    EC = ec_req
    NPAD = ((npad_req + 511) // 512) * 512
    cap = EC * P

    per_core = []
    for c in range(CORES):
        tgt_u, inv_k, e_src, e_tloc, e_win, cnt, nodes = pre[c]
        U_n = len(nodes)
        nlut = np.full(N, -1, np.int64)
        nlut[nodes] = np.arange(U_n)
        e_srcloc = nlut[e_src]

        start = np.concatenate([[0], np.cumsum(cnt)[:-1]])
        within = np.arange(len(e_tloc)) - start[e_win]
        slots = e_win * cap + within

        esrc_flat = np.zeros(W * cap, np.int64)
        etcol_flat = np.full(W * cap, -1.0, np.float32)
        esrc_flat[slots] = e_srcloc
        etcol_flat[slots] = (e_tloc - e_win * P).astype(np.float32)

        etcol = etcol_flat.reshape(W, EC, P).transpose(2, 0, 1).reshape(P, W * EC)
        etrow = etcol_flat.astype(BF16).reshape(1, W * cap)
        eidx = np.concatenate(
            [_wrap16(esrc_flat[w * cap:(w + 1) * cap]) for w in range(W)], axis=1)

        kvals = np.zeros(TP, np.int64)
        kvals[:KC] = inv_k
        kidx = _wrap16(kvals)

        xdt = BF16 if os.environ.get("KXBF", "1") == "1" else np.float32
        xT = np.zeros((IN, NPAD), xdt)
        xT[:, :U_n] = x[nodes].T
        per_core.append(dict(xT=xT, eidx=eidx, etcol=etcol,
                             etrow=etrow, kidx=kidx))
    return per_core, NPAD, EC


def make_weights(w_in, b_in, w_proj, a_src, a_trg, w_skip):
    w_in = np.asarray(w_in, np.float32)
    b_in = np.asarray(b_in, np.float32)
    w_proj = np.asarray(w_proj, np.float32)
    a_src = np.asarray(a_src, np.float32).reshape(NH, HD)
    a_trg = np.asarray(a_trg, np.float32).reshape(NH, HD)
    w_skip = np.asarray(w_skip, np.float32)

    wdt = BF16 if os.environ.get("KXBF", "1") == "1" else np.float32
    w1T = np.ascontiguousarray(w_in.T).astype(wdt)         # [256,128]
    b1 = b_in.reshape(H, 1).astype(np.float32)
    # B_src[h, a] = sum_d w_proj[a*16+d, h] * a_src[a, d]
    wp3 = w_proj.reshape(NH, HD, H)
    B_src = np.einsum("adh,ad->ha", wp3, a_src).astype(np.float32)  # [128,8]
    B_trg = np.einsum("adh,ad->ha", wp3, a_trg).astype(np.float32)
    w2 = np.zeros((H, 256), np.float32)
    w2[:, :OC] = w_proj.T
    w2[:, OC:OC + NH] = B_src
    wskT = np.ascontiguousarray(w_skip.T)                  # [128,128]
    iota4 = np.tile(np.arange(P, dtype=BF16)[None, :], (P, 4))
    iota_c = np.arange(P, dtype=np.float32).reshape(P, 1)
    ones1 = np.ones((1, P), BF16)
    return dict(w1T=w1T, b1=b1, w2=w2, wskT=wskT, btrg=B_trg,
                iota4=iota4, iota_c=iota_c, ones1=ones1)


# ----------------------------------------------------------------------------
# bass kernel
# ----------------------------------------------------------------------------

_BUILD_CACHE = {}


XBF = os.environ.get("KXBF", "1") == "1"


def build(NPAD, EC):
    PARTS = int(os.environ.get("KPARTS", "5"))
    key = (NPAD, EC, PARTS, XBF)
    if key in _BUILD_CACHE:
        return _BUILD_CACHE[key]

    import concourse.bacc as bacc
    import concourse.mybir as mybir
    import concourse.tile as tile

    dt = mybir.dt
    F32 = dt.float32
    F32R = dt.float32r
    I16 = dt.int16
    BF = dt.bfloat16
    AF = mybir.ActivationFunctionType
    OP = mybir.AluOpType

    NT = NPAD // 512
    cap = EC * P

    nc = bacc.Bacc("TRN2", target_bir_lowering=False)

    with tile.TileContext(nc) as tc:
        with tc.tile_pool(name="dram", bufs=1, space="DRAM") as dram:
            def din(name, shape, dtp):
                return dram.tile(shape, dtp, kind="ExternalInput", name=name,
                                 uniquify=False)

            XDT = BF if XBF else F32R
            xT = din("xT", [IN, NPAD], XDT)
            w1T = din("w1T", [IN, H], XDT)
            b1 = din("b1", [H, 1], F32)
            w2 = din("w2", [H, 256], F32R)
            wskT = din("wskT", [H, OC], F32R)
            btrg = din("btrg", [H, NH], F32R)
            eidx = din("eidx", [P, W * EC * 8], I16)
            etcol = din("etcol", [P, W * EC], F32)
            etrow = din("etrow", [1, W * cap], BF)
            kidx = din("kidx", [P, TP // 16], I16)
            iota4 = din("iota4", [P, 4 * P], BF)
            iota_c = din("iota_c", [P, 1], F32)
            ones1 = din("ones1", [1, P], BF)

            tabA = dram.tile([NPAD, 256], BF, kind="Internal", name="tabA",
                             uniquify=False)
            outT = dram.tile([TP, OC], F32, kind="Internal", name="outT",
                             uniquify=False)
            out = dram.tile([TP, OC], F32, kind="ExternalOutput", name="out",
                            uniquify=False)

        with tc.tile_pool(name="pers", bufs=1) as pers:
            w1a = pers.tile([P, H], XDT)
            w1b = pers.tile([P, H], XDT)
            b1s = pers.tile([H, 1], F32)
            w2s = pers.tile([H, 256], F32R)
            wsks = pers.tile([H, OC], F32R)
            btrgs = pers.tile([H, NH], F32R)
            iota4s = pers.tile([P, 4 * P], BF)
            iotac = pers.tile([P, 1], F32)
            ones1s = pers.tile([1, P], BF)
            hfmt = pers.tile([H, TP], F32R)       # targets' h, feature-major
            strg = pers.tile([P, W * NH], BF)     # per-window s_trg  [t, 8]
            eidxs = pers.tile([P, W * EC * 8], I16)
            etcols = pers.tile([P, W * EC], F32)
            etrows = pers.tile([1, W * cap], BF)
            kidxs = pers.tile([P, TP // 16], I16)

            nc.sync.dma_start(w1a[:], w1T[0:P, :])
            nc.sync.dma_start(w1b[:], w1T[P:IN, :])
            nc.sync.dma_start(b1s[:], b1[:])
            nc.sync.dma_start(w2s[:], w2[:])
            nc.sync.dma_start(wsks[:], wskT[:])
            nc.sync.dma_start(btrgs[:], btrg[:])
            nc.sync.dma_start(iota4s[:], iota4[:])
            nc.sync.dma_start(iotac[:], iota_c[:])
            nc.sync.dma_start(ones1s[:], ones1[:])
            nc.sync.dma_start(eidxs[:], eidx[:])
            nc.sync.dma_start(etcols[:], etcol[:])
            nc.sync.dma_start(etrows[:], etrow[:])
            nc.sync.dma_start(kidxs[:], kidx[:])

            # ---------------- phase A ----------------
            CH = 8  # 512-node tiles per xT load chunk
            with tc.tile_pool(name="pa", bufs=3) as pa, \
                 tc.tile_pool(name="pax", bufs=2) as pax, \
                 tc.tile_pool(name="pap", bufs=2, space="PSUM") as pap, \
                 tc.tile_pool(name="pap2", bufs=2, space="PSUM") as pap2:
                for t0 in range(0, NT, CH):
                    t1 = min(t0 + CH, NT)
                    wdc = (t1 - t0) * 512
                    slc = slice(t0 * 512, t0 * 512 + wdc)
                    xa = pax.tile([P, CH * 512], XDT, tag="xa")
                    nc.sync.dma_start(xa[:, 0:wdc], xT[0:P, slc])
                    xb = pax.tile([P, CH * 512], XDT, tag="xb")
                    nc.sync.dma_start(xb[:, 0:wdc], xT[P:IN, slc])
                    for t in range(t0, t1):
                        o = (t - t0) * 512
                        hps = pap.tile([P, 512], F32, tag="hps")
                        nc.tensor.matmul(hps[:], lhsT=w1a[:],
                                         rhs=xa[:, o:o + 512],
                                         start=True, stop=False)
                        nc.tensor.matmul(hps[:], lhsT=w1b[:],
                                         rhs=xb[:, o:o + 512],
                                         start=False, stop=True)
                        hsb = pa.tile([P, 512], F32R, tag="hsb")
                        nc.scalar.activation(hsb[:], hps[:], AF.Relu,
                                             bias=b1s[:])
                        if t * 512 < TP:
                            w0 = t * 512
                            w1_ = min(TP, (t + 1) * 512)
                            nc.scalar.activation(hfmt[:, w0:w1_],
                                                 hps[:, 0:(w1_ - w0)], AF.Relu,
                                                 bias=b1s[:])
                        stg = pa.tile([P, 4, 256], BF, tag="stg")
                        p2 = pap2.tile([P, 4, 256], F32, tag="p2")
                        for j in range(4):
                            nc.tensor.matmul(p2[:, j, :],
                                             lhsT=hsb[:, j * P:(j + 1) * P],
                                             rhs=w2s[:], start=True, stop=True)
                        nc.scalar.activation(stg[:, :, 0:OC],
                                             p2[:, :, 0:OC], AF.Copy)
                        # s_src f32 bits + zero pad tail in one copy (w2 cols
                        # 136:192 are zero so psum cols 136:192 are zero too)
                        nc.vector.tensor_copy(
                            stg[:, :, OC:256].bitcast(F32),
                            p2[:, :, OC:OC + 64])
                        r0 = t * 512
                        nc.sync.dma_start(
                            tabA[r0:r0 + 512, :].rearrange(
                                "(j p) f -> p j f", p=P), stg[:])

            # per-window s_trg from resident target h
            with tc.tile_pool(name="pstg", bufs=2, space="PSUM") as pstg:
                for w in range(W):
                    stp = pstg.tile([P, NH], F32, tag="stp")
                    nc.tensor.matmul(stp[:], lhsT=hfmt[:, w * P:(w + 1) * P],
                                     rhs=btrgs[:], start=True, stop=True)
                    nc.vector.tensor_copy(strg[:, w * NH:(w + 1) * NH], stp[:])

            # ---------------- phase B: edge windows ----------------
            SL = (EC + 3) // 4
            with tc.tile_pool(name="pe", bufs=3) as pe, \
                 tc.tile_pool(name="peg", bufs=3) as peg, \
                 tc.tile_pool(name="pep", bufs=2, space="PSUM") as pep, \
                 tc.tile_pool(name="pesk", bufs=2, space="PSUM") as pesk:
                # loop 1: expand s_trg to edge slots (independent of gathers,
                # overlaps the tail of phase A)
                st_sb = pers.tile([P, W, EC, NH], F32)
                for w in range(W if PARTS >= 3 else 0):
                    stps = pep.tile([P, EC, NH], F32, tag="stps")
                    for s in range(SL):
                        c0 = s * 4
                        c1 = min(c0 + 4, EC)
                        wd = (c1 - c0) * P
                        bc = pep.tile([P, 512], F32, tag="bc")
                        nc.tensor.matmul(
                            bc[:, 0:wd], lhsT=ones1s[:],
                            rhs=etrows[0:1, w * cap + c0 * P:w * cap + c0 * P + wd],
                            start=True, stop=True)
                        Mt = pe.tile([P, 512], BF, tag="Mt")
                        nc.vector.tensor_scalar(Mt[:, 0:wd], bc[:, 0:wd],
                                                iotac[:], None, OP.is_equal)
                        for j in range(c0, c1):
                            nc.tensor.matmul(
                                stps[:, j, :],
                                lhsT=Mt[:, (j - c0) * P:(j - c0 + 1) * P],
                                rhs=strg[:, w * NH:(w + 1) * NH],
                                start=True, stop=True)
                    nc.vector.tensor_copy(st_sb[:, w], stps[:])

                # loop 2: gather-dependent per-window pipeline
                for w in range(W if PARTS >= 2 else 0):
                    G = peg.tile([P, EC, 256], BF, tag="G")
                    nc.gpsimd.dma_gather(
                        G[:], tabA[:], eidxs[:, w * EC * 8:(w + 1) * EC * 8],
                        cap, cap, 256, single_packet=False)

                    if PARTS < 3:
                        continue
                    skp = pesk.tile([P, OC], F32, tag="skp")
                    nc.tensor.matmul(skp[:], lhsT=hfmt[:, w * P:(w + 1) * P],
                                     rhs=wsks[:], start=True, stop=True)

                    Mw = pe.tile([P, EC * P], BF, tag="Mw")
                    for j in range(EC):
                        nc.vector.tensor_scalar(
                            Mw[:, j * P:(j + 1) * P], iota4s[:, 0:P],
                            etcols[:, w * EC + j:w * EC + j + 1], None,
                            OP.is_equal)

                    # scores = s_src(gathered) + s_trg(expanded)
                    sc = pe.tile([P, EC, NH], F32, tag="sc")
                    gss = G[:, :, OC:OC + 2 * NH].bitcast(F32)
                    nc.vector.tensor_tensor(sc[:], st_sb[:, w], gss, OP.add)
                    # exp(leakyrelu(s)) = max(exp(s), exp(0.2 s))  (monotonic)
                    e1 = pe.tile([P, EC, NH], BF, tag="e1")
                    nc.scalar.activation(e1[:], sc[:], AF.Exp)
                    e2 = pe.tile([P, EC, NH], BF, tag="e2")
                    nc.scalar.activation(e2[:], sc[:], AF.Exp, scale=0.2)
                    emax = pe.tile([P, EC, NH], BF, tag="emax")
                    nc.vector.tensor_max(emax[:], e1[:], e2[:])
                    Wv = pe.tile([P, EC, 136], BF, tag="Wv")
                    nc.vector.tensor_copy(Wv[:, :, OC:OC + NH], emax[:])
                    # expand exp per-head on ACT (broadcast read), then a fully
                    # packed bf16 multiply on DVE (2x mode)
                    eex = pe.tile([P, EC, OC], BF, tag="eex")
                    nc.scalar.activation(
                        eex[:].rearrange("p j (a d) -> p j a d", d=HD),
                        emax[:].broadcast_to([P, EC, NH, HD]), AF.Copy)
                    nc.vector.tensor_tensor(Wv[:, :, 0:OC], G[:, :, 0:OC],
                                            eex[:], OP.mult)

                    if PARTS < 4:
                        continue
                    segp = pep.tile([P, 136], F32, tag="segp")
                    for j in range(EC):
                        nc.tensor.matmul(segp[:], lhsT=Mw[:, j * P:(j + 1) * P],
                                         rhs=Wv[:, j, :], start=(j == 0),
                                         stop=(j == EC - 1))

                    den = pe.tile([P, NH], F32, tag="den")
                    nc.vector.tensor_scalar_add(den[:], segp[:, OC:OC + NH], EPS)
                    rec = pe.tile([P, NH], F32, tag="rec")
                    nc.vector.reciprocal(rec[:], den[:])
                    z = pe.tile([P, OC], F32, tag="z")
                    recb = rec[:].broadcast_to([P, NH, HD])
                    nc.vector.tensor_tensor(
                        z[:].rearrange("p (a d) -> p a d", d=HD),
                        segp[:, 0:OC].rearrange("p (a d) -> p a d", d=HD),
                        recb, OP.mult)
                    nc.vector.tensor_add(z[:], z[:], skp[:])
                    # elu: max(z,0)-1 + exp(min(z,0))
                    am = pe.tile([P, OC], F32, tag="am")
                    nc.vector.tensor_scalar(am[:], z[:], 0.0, -1.0, OP.max,
                                            OP.add)
                    bm = pe.tile([P, OC], F32, tag="bm")
                    nc.vector.tensor_scalar(bm[:], z[:], 0.0, None, OP.min)
                    eb = pe.tile([P, OC], F32, tag="eb")
                    nc.scalar.activation(eb[:], bm[:], AF.Exp)
                    fo = pe.tile([P, OC], F32, tag="fo")
                    nc.vector.tensor_add(fo[:], am[:], eb[:])
                    nc.sync.dma_start(outT[w * P:(w + 1) * P, :], fo[:])

                # final k-row gather
                if PARTS >= 5:
                    ko = peg.tile([P, TP // P, OC], F32, tag="ko")
                    nc.gpsimd.dma_gather(ko[:], outT[:], kidxs[:], TP, TP, OC,
                                         single_packet=False)
                    nc.sync.dma_start(
                        out[:].rearrange("(j p) f -> p j f", p=P), ko[:])

    nc.compile()
    _BUILD_CACHE[key] = nc
    return nc


# ----------------------------------------------------------------------------
# entry point
# ----------------------------------------------------------------------------

def kernel(x, adj0, index0, w_in, b_in, w_proj, a_src, a_trg, w_skip):
    from concourse.bass_utils import run_bass_kernel_spmd

    per_core, NPAD, EC = plan(x, adj0, index0)
    wts = make_weights(w_in, b_in, w_proj, a_src, a_trg, w_skip)
    nc = build(NPAD, EC)

    in_maps = []
    for c in range(CORES):
        m = dict(wts)
        pc = per_core[c]
        m.update(pc)
        in_maps.append(m)

    res = run_bass_kernel_spmd(nc, in_maps, core_ids=list(range(CORES)))
    outs = [r["out"][:KC] for r in res.results]
    return np.concatenate(outs, axis=0).astype(np.float32)


# revision 19
# speedup vs baseline: 1.0736x; 1.0736x over previous
"""GAT BasicAttentionBlock kernel for 8x Trainium2 NeuronCores.

Strategy (output-shard): each core owns 1250 of the 10000 selected output
rows (index0).  Only nodes reachable from those rows matter: ~1.2k unique
target nodes and ~16k unique source nodes per core (~5.5x less edge work
than the full graph).  Per core:

  phase A  gather x rows of needed nodes (host "halo"), compute
           h = relu(x@w1.T + b) feature-major on PE, then per 128-node
           subtile proj = h@w_proj.T and s_src = h@B_src node-major into a
           512-byte/row HBM table  [proj bf16 128 | s_src f32 8 | pad].
  loop 1   per 128-target window: s_trg/skip for the window targets from
           the resident h, and the edge-slot expansion of s_trg via a
           one-hot matmul (overlaps phase A's DMA tail).
  loop 2   per window: dma_gather the table rows of each edge's source
           (512B granules), scores = leakyrelu(s_src+s_trg) via
           exp(lrelu(s)) = max(exp(s), exp(0.2s)), weighted = exp * proj,
           segment-sum via one-hot matmuls accumulated in PSUM:
           out[t] = [sum exp*proj | sum exp], out = att/den + skip, ELU.
  final    dma_gather the 1250 output rows from the 1280-target table.

No collectives: cores are fully independent.  The softmax max-subtraction
in the reference cancels in the att = exp/sum(exp) ratio and is dropped
(scores are O(1) here, exp cannot overflow).
"""

import os
import sys

for _p in ("/opt/trn_rl_repo",):
    if os.path.isdir(_p) and _p not in sys.path:
        sys.path.insert(0, _p)

import numpy as np
import ml_dtypes

# problem constants (hardcoded per contract)
N = 50000
E = 800000
K = 10000
IN = 256
H = 128
NH = 8
HD = 16
OC = NH * HD  # 128
CORES = 8
KC = K // CORES          # 1250 output rows per core
P = 128
W = 10                   # target windows of 128 -> 1280 target slots
TP = W * P               # padded target count per core
EPS = 1e-16

BF16 = ml_dtypes.bfloat16

XBF = os.environ.get("KXBF", "1") == "1"


# ----------------------------------------------------------------------------
# host-side sharding / planning
# ----------------------------------------------------------------------------

def _wrap16(vals, reps=8):
    """int16 index layout for dma_gather: idx i at [i%16, i//16], the 16-row
    block replicated `reps` times down the partition axis."""
    L = vals.shape[0]
    assert L % 16 == 0
    w = vals.reshape(L // 16, 16).T.astype(np.int16)
    return np.tile(w, (reps, 1))


def plan(x, adj0, index0):
    src_all = np.asarray(adj0[0], dtype=np.int64)
    trg_all = np.asarray(adj0[1], dtype=np.int64)
    idx0 = np.asarray(index0, dtype=np.int64)
    x = np.asarray(x, dtype=np.float32)

    pre = []
    ec_req = 1
    npad_req = 512
    for c in range(CORES):
        ks = idx0[c * KC:(c + 1) * KC]
        tgt_u, inv_k = np.unique(ks, return_inverse=True)
        U_t = len(tgt_u)
        assert U_t <= TP
        lut = np.full(N, -1, np.int64)
        lut[tgt_u] = np.arange(U_t)
        tloc_all = lut[trg_all]
        sel = np.nonzero(tloc_all >= 0)[0]
        e_src = src_all[sel]
        e_tloc = tloc_all[sel]
        order = np.argsort(e_tloc, kind="stable")
        e_src = e_src[order]
        e_tloc = e_tloc[order]
        e_win = e_tloc >> 7
        cnt = np.bincount(e_win, minlength=W)
        ec_req = max(ec_req, int(np.ceil(cnt.max() / P)))

        extra = np.setdiff1d(np.unique(e_src), tgt_u)
        nodes = np.concatenate([tgt_u, extra])
        npad_req = max(npad_req, len(nodes))
        pre.append((tgt_u, inv_k, e_src, e_tloc, e_win, cnt, nodes))

    EC = ec_req
    NPAD = ((npad_req + 511) // 512) * 512
    cap = EC * P

    per_core = []
    for c in range(CORES):
        tgt_u, inv_k, e_src, e_tloc, e_win, cnt, nodes = pre[c]
        U_n = len(nodes)
        nlut = np.full(N, -1, np.int64)
        nlut[nodes] = np.arange(U_n)
        e_srcloc = nlut[e_src]

        start = np.concatenate([[0], np.cumsum(cnt)[:-1]])
        within = np.arange(len(e_tloc)) - start[e_win]
        slots = e_win * cap + within

        esrc_flat = np.zeros(W * cap, np.int64)
        etcol_flat = np.full(W * cap, -1.0, np.float32)
        esrc_flat[slots] = e_srcloc
        etcol_flat[slots] = (e_tloc - e_win * P).astype(np.float32)

        etcol = etcol_flat.reshape(W, EC, P).transpose(2, 0, 1).reshape(P, W * EC)
        etrow = etcol_flat.astype(BF16).reshape(1, W * cap)
        eidx = np.concatenate(
            [_wrap16(esrc_flat[w * cap:(w + 1) * cap]) for w in range(W)], axis=1)

        kvals = np.zeros(TP, np.int64)
        kvals[:KC] = inv_k
        kidx = _wrap16(kvals)

        xdt = BF16 if XBF else np.float32
        xT = np.zeros((IN, NPAD), xdt)
        xT[:, :U_n] = x[nodes].T

        per_core.append(dict(xT=xT, eidx=eidx, etcol=etcol,
                             etrow=etrow, kidx=kidx))
    return per_core, NPAD, EC


def make_weights(w_in, b_in, w_proj, a_src, a_trg, w_skip):
    w_in = np.asarray(w_in, np.float32)
    b_in = np.asarray(b_in, np.float32)
    w_proj = np.asarray(w_proj, np.float32)
    a_src = np.asarray(a_src, np.float32).reshape(NH, HD)
    a_trg = np.asarray(a_trg, np.float32).reshape(NH, HD)
    w_skip = np.asarray(w_skip, np.float32)

    wdt = BF16 if XBF else np.float32
    w1T = np.ascontiguousarray(w_in.T).astype(wdt)         # [256,128]
    b1 = b_in.reshape(H, 1).astype(np.float32)
    # B_src[h, a] = sum_d w_proj[a*16+d, h] * a_src[a, d]
    wp3 = w_proj.reshape(NH, HD, H)
    B_src = np.einsum("adh,ad->ha", wp3, a_src).astype(np.float32)  # [128,8]
    B_trg = np.einsum("adh,ad->ha", wp3, a_trg).astype(np.float32)
    w2 = np.zeros((H, 256), np.float32)
    w2[:, :OC] = w_proj.T
    w2[:, OC:OC + NH] = B_src
    wskT = np.ascontiguousarray(w_skip.T)                  # [128,128]
    iota4 = np.tile(np.arange(P, dtype=BF16)[None, :], (P, 4))
    iota_c = np.arange(P, dtype=np.float32).reshape(P, 1)
    return dict(w1T=w1T, b1=b1, w2=w2, wskT=wskT, btrg=B_trg,
                iota4=iota4, iota_c=iota_c)


# ----------------------------------------------------------------------------
# bass kernel
# ----------------------------------------------------------------------------

_BUILD_CACHE = {}


def build(NPAD, EC):
    PARTS = int(os.environ.get("KPARTS", "5"))
    key = (NPAD, EC, PARTS, XBF)
    if key in _BUILD_CACHE:
        return _BUILD_CACHE[key]

    import concourse.bacc as bacc
    import concourse.mybir as mybir
    import concourse.tile as tile

    dt = mybir.dt
    F32 = dt.float32
    F32R = dt.float32r
    I16 = dt.int16
    BF = dt.bfloat16
    AF = mybir.ActivationFunctionType
    OP = mybir.AluOpType

    NT = NPAD // 512
    cap = EC * P

    nc = bacc.Bacc("TRN2", target_bir_lowering=False)

    with tile.TileContext(nc) as tc:
        with tc.tile_pool(name="dram", bufs=1, space="DRAM") as dram:
            def din(name, shape, dtp):
                return dram.tile(shape, dtp, kind="ExternalInput", name=name,
                                 uniquify=False)

            XDT = BF if XBF else F32R
            xT = din("xT", [IN, NPAD], XDT)
            w1T = din("w1T", [IN, H], XDT)
            b1 = din("b1", [H, 1], F32)
            w2 = din("w2", [H, 256], F32R)
            wskT = din("wskT", [H, OC], F32R)
            btrg = din("btrg", [H, NH], F32R)
            eidx = din("eidx", [P, W * EC * 8], I16)
            etcol = din("etcol", [P, W * EC], F32)
            etrow = din("etrow", [1, W * cap], BF)
            kidx = din("kidx", [P, TP // 16], I16)
            iota4 = din("iota4", [P, 4 * P], BF)
            iota_c = din("iota_c", [P, 1], F32)

            tabA = dram.tile([NPAD, 256], BF, kind="Internal", name="tabA",
                             uniquify=False)
            outT = dram.tile([TP, OC], F32, kind="Internal", name="outT",
                             uniquify=False)
            out = dram.tile([TP, OC], F32, kind="ExternalOutput", name="out",
                            uniquify=False)

        with tc.tile_pool(name="pers", bufs=1) as pers:
            w1a = pers.tile([P, H], XDT)
            w1b = pers.tile([P, H], XDT)
            b1s = pers.tile([H, 1], F32)
            w2s = pers.tile([H, 256], F32R)
            wsks = pers.tile([H, OC], F32R)
            btrgs = pers.tile([H, NH], F32R)
            iota4s = pers.tile([P, 4 * P], BF)
            iotac = pers.tile([P, 1], F32)
            hfmt = pers.tile([H, TP], F32R)       # targets' h, feature-major
            strg = pers.tile([P, W * NH], BF)     # per-window s_trg  [t, 8]
            skips = pers.tile([P, W, OC], F32)    # per-window skip   [t, oc]
            st_sb = pers.tile([P, W, EC, NH], F32)  # s_trg per edge slot
            eidxs = pers.tile([P, W * EC * 8], I16)
            etcols = pers.tile([P, W * EC], F32)
            kidxs = pers.tile([P, TP // 16], I16)

            nc.sync.dma_start(w1a[:], w1T[0:P, :])
            nc.sync.dma_start(w1b[:], w1T[P:IN, :])
            nc.sync.dma_start(b1s[:], b1[:])
            nc.sync.dma_start(w2s[:], w2[:])
            nc.sync.dma_start(wsks[:], wskT[:])
            nc.sync.dma_start(btrgs[:], btrg[:])
            nc.sync.dma_start(iota4s[:], iota4[:])
            nc.sync.dma_start(iotac[:], iota_c[:])
            nc.sync.dma_start(eidxs[:], eidx[:])
            nc.sync.dma_start(etcols[:], etcol[:])
            nc.sync.dma_start(kidxs[:], kidx[:])

            CH = 8  # 512-node tiles per xT load chunk
            SL = (EC + 3) // 4
            with tc.tile_pool(name="pa", bufs=3) as pa, \
                 tc.tile_pool(name="pax", bufs=2) as pax, \
                 tc.tile_pool(name="pe", bufs=2) as pe, \
                 tc.tile_pool(name="peg", bufs=3) as peg, \
                 tc.tile_pool(name="psa", bufs=2, space="PSUM") as psa, \
                 tc.tile_pool(name="psb", bufs=2, space="PSUM") as psb, \
                 tc.tile_pool(name="psc", bufs=1, space="PSUM") as psc, \
                 tc.tile_pool(name="psd", bufs=1, space="PSUM") as psd, \
                 tc.tile_pool(name="pse", bufs=2, space="PSUM") as pse:

                # ---------------- phase A ----------------
                for t0 in range(0, NT, CH):
                    t1 = min(t0 + CH, NT)
                    wdc = (t1 - t0) * 512
                    slc = slice(t0 * 512, t0 * 512 + wdc)
                    xa = pax.tile([P, CH * 512], XDT, tag="xa")
                    nc.sync.dma_start(xa[:, 0:wdc], xT[0:P, slc])
                    xb = pax.tile([P, CH * 512], XDT, tag="xb")
                    nc.sync.dma_start(xb[:, 0:wdc], xT[P:IN, slc])
                    for t in range(t0, t1):
                        o = (t - t0) * 512
                        hps = psa.tile([P, 512], F32, tag="hps")
                        nc.tensor.matmul(hps[:], lhsT=w1a[:],
                                         rhs=xa[:, o:o + 512],
                                         start=True, stop=False)
                        nc.tensor.matmul(hps[:], lhsT=w1b[:],
                                         rhs=xb[:, o:o + 512],
                                         start=False, stop=True)
                        hsb = pa.tile([P, 512], F32R, tag="hsb")
                        nc.scalar.activation(hsb[:], hps[:], AF.Relu,
                                             bias=b1s[:])
                        if t * 512 < TP:
                            w0 = t * 512
                            w1_ = min(TP, (t + 1) * 512)
                            nc.scalar.activation(hfmt[:, w0:w1_],
                                                 hps[:, 0:(w1_ - w0)], AF.Relu,
                                                 bias=b1s[:])
                        stg = pa.tile([P, 4, 256], BF, tag="stg")
                        for half in range(2):
                            p2 = psb.tile([P, 2, 256], F32, tag="p2")
                            for jj in range(2):
                                j = half * 2 + jj
                                nc.tensor.matmul(
                                    p2[:, jj, :],
                                    lhsT=hsb[:, j * P:(j + 1) * P],
                                    rhs=w2s[:], start=True, stop=True)
                            sgh = stg[:, half * 2:half * 2 + 2, :]
                            nc.scalar.activation(sgh[:, :, 0:OC],
                                                 p2[:, :, 0:OC], AF.Copy)
                            # s_src f32 bits + zero pad tail in one copy (w2
                            # cols 136:192 are zero -> psum cols are zero)
                            nc.vector.tensor_copy(
                                sgh[:, :, OC:256].bitcast(F32),
                                p2[:, :, OC:OC + 64])
                        r0 = t * 512
                        nc.sync.dma_start(
                            tabA[r0:r0 + 512, :].rearrange(
                                "(j p) f -> p j f", p=P), stg[:])

                # ---------------- loop 1a: per-window s_trg / skip ----------
                for w in range(W):
                    stp = psd.tile([P, OC], F32, tag="misc")
                    nc.tensor.matmul(stp[:, 0:NH],
                                     lhsT=hfmt[:, w * P:(w + 1) * P],
                                     rhs=btrgs[:], start=True, stop=True)
                    nc.vector.tensor_copy(strg[:, w * NH:(w + 1) * NH],
                                          stp[:, 0:NH])
                    skp = psd.tile([P, OC], F32, tag="misc")
                    nc.tensor.matmul(skp[:], lhsT=hfmt[:, w * P:(w + 1) * P],
                                     rhs=wsks[:], start=True, stop=True)
                    nc.vector.tensor_copy(skips[:, w], skp[:])

                # ---------------- loop 1b: s_trg -> edge slots --------------
                for w in range(W if PARTS >= 3 else 0):
                    etw = pe.tile([1, cap], BF, tag="etw")
                    nc.sync.dma_start(etw[:], etrow[0:1, w * cap:(w + 1) * cap])
                    pbcw = pe.tile([P, cap], BF, tag="pbcw")
                    nc.gpsimd.partition_broadcast(pbcw[:], etw[:])
                    Mtw = pe.tile([P, cap], BF, tag="Mtw")
                    nc.vector.tensor_scalar(Mtw[:], pbcw[:], iotac[:], None,
                                            OP.is_equal)
                    stps = psc.tile([P, EC, NH], F32, tag="stps")
                    for j in range(EC):
                        nc.tensor.matmul(
                            stps[:, j, :], lhsT=Mtw[:, j * P:(j + 1) * P],
                            rhs=strg[:, w * NH:(w + 1) * NH],
                            start=True, stop=True)
                    nc.vector.tensor_copy(st_sb[:, w], stps[:])

                # ---------------- loop 2: per-window edge pipeline ----------
                for w in range(W if PARTS >= 2 else 0):
                    G = peg.tile([P, EC, 256], BF, tag="G")
                    nc.gpsimd.dma_gather(
                        G[:], tabA[:], eidxs[:, w * EC * 8:(w + 1) * EC * 8],
                        cap, cap, 256, single_packet=False)

                    if PARTS < 3:
                        continue
                    Mw = pe.tile([P, EC * P], BF, tag="Mw")
                    for j in range(EC):
                        nc.vector.tensor_scalar(
                            Mw[:, j * P:(j + 1) * P], iota4s[:, 0:P],
                            etcols[:, w * EC + j:w * EC + j + 1], None,
                            OP.is_equal)

                    # scores = s_src(gathered) + s_trg(expanded)
                    sc = pe.tile([P, EC, NH], F32, tag="sc")
                    gss = G[:, :, OC:OC + 2 * NH].bitcast(F32)
                    nc.vector.tensor_tensor(sc[:], st_sb[:, w], gss, OP.add)
                    # exp(leakyrelu(s)) = max(exp(s), exp(0.2 s))  (monotonic)
                    e1 = pe.tile([P, EC, NH], BF, tag="e1")
                    nc.scalar.activation(e1[:], sc[:], AF.Exp)
                    e2 = pe.tile([P, EC, NH], BF, tag="e2")
                    nc.scalar.activation(e2[:], sc[:], AF.Exp, scale=0.2)
                    emax = pe.tile([P, EC, NH], BF, tag="emax")
                    nc.vector.tensor_max(emax[:], e1[:], e2[:])
                    Wv = pe.tile([P, EC, 136], BF, tag="Wv")
                    nc.vector.tensor_copy(Wv[:, :, OC:OC + NH], emax[:])
                    # expand exp per-head on ACT (broadcast read), then a
                    # fully packed bf16 multiply on DVE (2x mode)
                    eex = pe.tile([P, EC, OC], BF, tag="eex")
                    nc.scalar.activation(
                        eex[:].rearrange("p j (a d) -> p j a d", d=HD),
                        emax[:].broadcast_to([P, EC, NH, HD]), AF.Copy)
                    nc.vector.tensor_tensor(Wv[:, :, 0:OC], G[:, :, 0:OC],
                                            eex[:], OP.mult)

                    if PARTS < 4:
                        continue
                    segp = pse.tile([P, 136], F32, tag="segp")
                    for j in range(EC):
                        nc.tensor.matmul(segp[:], lhsT=Mw[:, j * P:(j + 1) * P],
                                         rhs=Wv[:, j, :], start=(j == 0),
                                         stop=(j == EC - 1))

                    den = pe.tile([P, NH], F32, tag="den")
                    nc.vector.tensor_scalar_add(den[:], segp[:, OC:OC + NH], EPS)
                    rec = pe.tile([P, NH], F32, tag="rec")
                    nc.vector.reciprocal(rec[:], den[:])
                    z = pe.tile([P, OC], F32, tag="z")
                    recb = rec[:].broadcast_to([P, NH, HD])
                    nc.vector.tensor_tensor(
                        z[:].rearrange("p (a d) -> p a d", d=HD),
                        segp[:, 0:OC].rearrange("p (a d) -> p a d", d=HD),
                        recb, OP.mult)
                    nc.vector.tensor_add(z[:], z[:], skips[:, w])
                    # elu: (max(z,0)-1) + exp(min(z,0))
                    am = pe.tile([P, OC], F32, tag="am")
                    nc.vector.tensor_scalar(am[:], z[:], 0.0, -1.0, OP.max,
                                            OP.add)
                    bm = pe.tile([P, OC], F32, tag="bm")
                    nc.vector.tensor_scalar(bm[:], z[:], 0.0, None, OP.min)
                    eb = pe.tile([P, OC], F32, tag="eb")
                    nc.scalar.activation(eb[:], bm[:], AF.Exp)
                    fo = pe.tile([P, OC], F32, tag="fo")
                    nc.vector.tensor_add(fo[:], am[:], eb[:])
                    nc.sync.dma_start(outT[w * P:(w + 1) * P, :], fo[:])

                # final k-row gather
                if PARTS >= 5:
                    ko = peg.tile([P, TP // P, OC], F32, tag="ko")
                    nc.gpsimd.dma_gather(ko[:], outT[:], kidxs[:], TP, TP, OC,
                                         single_packet=False)
                    nc.sync.dma_start(
                        out[:].rearrange("(j p) f -> p j f", p=P), ko[:])

    nc.compile()
    _BUILD_CACHE[key] = nc
    return nc


# ----------------------------------------------------------------------------
# entry point
# ----------------------------------------------------------------------------

def kernel(x, adj0, index0, w_in, b_in, w_proj, a_src, a_trg, w_skip):
    from concourse.bass_utils import run_bass_kernel_spmd

    per_core, NPAD, EC = plan(x, adj0, index0)
    wts = make_weights(w_in, b_in, w_proj, a_src, a_trg, w_skip)
    nc = build(NPAD, EC)

    in_maps = []
    for c in range(CORES):
        m = dict(wts)
        m.update(per_core[c])
        in_maps.append(m)

    res = run_bass_kernel_spmd(nc, in_maps, core_ids=list(range(CORES)))
    outs = [r["out"][:KC] for r in res.results]
    return np.concatenate(outs, axis=0).astype(np.float32)


# revision 20
# speedup vs baseline: 1.1547x; 1.0756x over previous
"""GAT BasicAttentionBlock kernel for 8x Trainium2 NeuronCores.

Strategy (output-shard): each core owns 1250 of the 10000 selected output
rows (index0).  Only nodes reachable from those rows matter: ~1.2k unique
target nodes and ~16k unique source nodes per core (~5.5x less edge work
than the full graph).  Per core:

  phase A  gather x rows of needed nodes (host "halo"), compute
           h = relu(x@w1.T + b) feature-major on PE, then per 128-node
           subtile proj = h@w_proj.T and s_src = h@B_src node-major into a
           512-byte/row HBM table  [proj bf16 128 | s_src f32 8 | pad].
  loop 1   per 128-target window: s_trg/skip for the window targets from
           the resident h, and the edge-slot expansion of s_trg via a
           one-hot matmul (overlaps phase A's DMA tail).
  loop 2   per window: dma_gather the table rows of each edge's source
           (512B granules), scores = leakyrelu(s_src+s_trg) via
           exp(lrelu(s)) = max(exp(s), exp(0.2s)), weighted = exp * proj,
           segment-sum via one-hot matmuls accumulated in PSUM:
           out[t] = [sum exp*proj | sum exp], out = att/den + skip, ELU.
  final    dma_gather the 1250 output rows from the 1280-target table.

No collectives: cores are fully independent.  The softmax max-subtraction
in the reference cancels in the att = exp/sum(exp) ratio and is dropped
(scores are O(1) here, exp cannot overflow).
"""

import os
import sys

for _p in ("/opt/trn_rl_repo",):
    if os.path.isdir(_p) and _p not in sys.path:
        sys.path.insert(0, _p)

import numpy as np
import ml_dtypes

# problem constants (hardcoded per contract)
N = 50000
E = 800000
K = 10000
IN = 256
H = 128
NH = 8
HD = 16
OC = NH * HD  # 128
CORES = 8
KC = K // CORES          # 1250 output rows per core
P = 128
W = 10                   # target windows of 128 -> 1280 target slots
TP = W * P               # padded target count per core
EPS = 1e-16

BF16 = ml_dtypes.bfloat16

XBF = os.environ.get("KXBF", "1") == "1"


# ----------------------------------------------------------------------------
# host-side sharding / planning
# ----------------------------------------------------------------------------

def _wrap16(vals, reps=8):
    """int16 index layout for dma_gather: idx i at [i%16, i//16], the 16-row
    block replicated `reps` times down the partition axis."""
    L = vals.shape[0]
    assert L % 16 == 0
    w = vals.reshape(L // 16, 16).T.astype(np.int16)
    return np.tile(w, (reps, 1))


def plan(x, adj0, index0):
    src_all = np.asarray(adj0[0], dtype=np.int64)
    trg_all = np.asarray(adj0[1], dtype=np.int64)
    idx0 = np.asarray(index0, dtype=np.int64)
    x = np.asarray(x, dtype=np.float32)

    pre = []
    ec_req = 1
    npad_req = 512
    for c in range(CORES):
        ks = idx0[c * KC:(c + 1) * KC]
        tgt_u, inv_k = np.unique(ks, return_inverse=True)
        U_t = len(tgt_u)
        assert U_t <= TP
        lut = np.full(N, -1, np.int64)
        lut[tgt_u] = np.arange(U_t)
        tloc_all = lut[trg_all]
        sel = np.nonzero(tloc_all >= 0)[0]
        e_src = src_all[sel]
        e_tloc = tloc_all[sel]
        order = np.argsort(e_tloc, kind="stable")
        e_src = e_src[order]
        e_tloc = e_tloc[order]
        e_win = e_tloc >> 7
        cnt = np.bincount(e_win, minlength=W)
        ec_req = max(ec_req, int(np.ceil(cnt.max() / P)))

        extra = np.setdiff1d(np.unique(e_src), tgt_u)
        nodes = np.concatenate([tgt_u, extra])
        npad_req = max(npad_req, len(nodes))
        pre.append((tgt_u, inv_k, e_src, e_tloc, e_win, cnt, nodes))

    EC = ec_req
    NPAD = ((npad_req + 511) // 512) * 512
    cap = EC * P

    per_core = []
    for c in range(CORES):
        tgt_u, inv_k, e_src, e_tloc, e_win, cnt, nodes = pre[c]
        U_n = len(nodes)
        nlut = np.full(N, -1, np.int64)
        nlut[nodes] = np.arange(U_n)
        e_srcloc = nlut[e_src]

        start = np.concatenate([[0], np.cumsum(cnt)[:-1]])
        within = np.arange(len(e_tloc)) - start[e_win]
        slots = e_win * cap + within

        esrc_flat = np.zeros(W * cap, np.int64)
        etcol_flat = np.full(W * cap, -1.0, np.float32)
        esrc_flat[slots] = e_srcloc
        etcol_flat[slots] = (e_tloc - e_win * P).astype(np.float32)

        etcol = etcol_flat.reshape(W, EC, P).transpose(2, 0, 1).reshape(P, W * EC)
        etrow = etcol_flat.astype(BF16).reshape(1, W * cap)
        eidx = np.concatenate(
            [_wrap16(esrc_flat[w * cap:(w + 1) * cap]) for w in range(W)], axis=1)

        kvals = np.zeros(TP, np.int64)
        kvals[:KC] = inv_k
        kidx = _wrap16(kvals)

        xdt = BF16 if XBF else np.float32
        xT = np.zeros((IN, NPAD), xdt)
        xT[:, :U_n] = x[nodes].T

        per_core.append(dict(xT=xT, eidx=eidx, etcol=etcol,
                             etrow=etrow, kidx=kidx))
    return per_core, NPAD, EC


def make_weights(w_in, b_in, w_proj, a_src, a_trg, w_skip):
    w_in = np.asarray(w_in, np.float32)
    b_in = np.asarray(b_in, np.float32)
    w_proj = np.asarray(w_proj, np.float32)
    a_src = np.asarray(a_src, np.float32).reshape(NH, HD)
    a_trg = np.asarray(a_trg, np.float32).reshape(NH, HD)
    w_skip = np.asarray(w_skip, np.float32)

    wdt = BF16 if XBF else np.float32
    w1T = np.ascontiguousarray(w_in.T).astype(wdt)         # [256,128]
    b1 = b_in.reshape(H, 1).astype(np.float32)
    # B_src[h, a] = sum_d w_proj[a*16+d, h] * a_src[a, d]
    wp3 = w_proj.reshape(NH, HD, H)
    B_src = np.einsum("adh,ad->ha", wp3, a_src).astype(np.float32)  # [128,8]
    B_trg = np.einsum("adh,ad->ha", wp3, a_trg).astype(np.float32)
    w2 = np.zeros((H, 256), np.float32)
    w2[:, :OC] = w_proj.T
    w2[:, OC:OC + NH] = B_src
    wskT = np.ascontiguousarray(w_skip.T)                  # [128,128]
    iota4 = np.tile(np.arange(P, dtype=BF16)[None, :], (P, 4))
    iota_c = np.arange(P, dtype=np.float32).reshape(P, 1)
    return dict(w1T=w1T, b1=b1, w2=w2, wskT=wskT, btrg=B_trg,
                iota4=iota4, iota_c=iota_c)


# ----------------------------------------------------------------------------
# bass kernel
# ----------------------------------------------------------------------------

_BUILD_CACHE = {}


def build(NPAD, EC):
    PARTS = int(os.environ.get("KPARTS", "5"))
    key = (NPAD, EC, PARTS, XBF)
    if key in _BUILD_CACHE:
        return _BUILD_CACHE[key]

    import concourse.bacc as bacc
    import concourse.mybir as mybir
    import concourse.tile as tile

    dt = mybir.dt
    F32 = dt.float32
    F32R = dt.float32r
    I16 = dt.int16
    BF = dt.bfloat16
    AF = mybir.ActivationFunctionType
    OP = mybir.AluOpType

    NT = NPAD // 512
    cap = EC * P

    nc = bacc.Bacc("TRN2", target_bir_lowering=False)

    with tile.TileContext(nc) as tc:
        with tc.tile_pool(name="dram", bufs=1, space="DRAM") as dram:
            def din(name, shape, dtp):
                return dram.tile(shape, dtp, kind="ExternalInput", name=name,
                                 uniquify=False)

            XDT = BF if XBF else F32R
            xT = din("xT", [IN, NPAD], XDT)
            w1T = din("w1T", [IN, H], XDT)
            b1 = din("b1", [H, 1], F32)
            w2 = din("w2", [H, 256], F32R)
            wskT = din("wskT", [H, OC], F32R)
            btrg = din("btrg", [H, NH], F32R)
            eidx = din("eidx", [P, W * EC * 8], I16)
            etcol = din("etcol", [P, W * EC], F32)
            etrow = din("etrow", [1, W * cap], BF)
            kidx = din("kidx", [P, TP // 16], I16)
            iota4 = din("iota4", [P, 4 * P], BF)
            iota_c = din("iota_c", [P, 1], F32)

            tabA = dram.tile([NPAD, 256], BF, kind="Internal", name="tabA",
                             uniquify=False)
            outT = dram.tile([TP, OC], F32, kind="Internal", name="outT",
                             uniquify=False)
            out = dram.tile([TP, OC], F32, kind="ExternalOutput", name="out",
                            uniquify=False)

        with tc.tile_pool(name="pers", bufs=1) as pers:
            w1a = pers.tile([P, H], XDT)
            w1b = pers.tile([P, H], XDT)
            b1s = pers.tile([H, 1], F32)
            w2s = pers.tile([H, 256], F32R)
            wsks = pers.tile([H, OC], F32R)
            btrgs = pers.tile([H, NH], F32R)
            iota4s = pers.tile([P, 4 * P], BF)
            iotac = pers.tile([P, 1], F32)
            hfmt = pers.tile([H, TP], F32R)       # targets' h, feature-major
            strg = pers.tile([P, W * NH], BF)     # per-window s_trg  [t, 8]
            skips = pers.tile([P, W, OC], F32)    # per-window skip   [t, oc]
            st_sb = pers.tile([P, W, EC, NH], F32)  # s_trg per edge slot
            eidxs = pers.tile([P, W * EC * 8], I16)
            etcols = pers.tile([P, W * EC], F32)
            kidxs = pers.tile([P, TP // 16], I16)

            nc.sync.dma_start(w1a[:], w1T[0:P, :])
            nc.sync.dma_start(w1b[:], w1T[P:IN, :])
            nc.sync.dma_start(b1s[:], b1[:])
            nc.sync.dma_start(w2s[:], w2[:])
            nc.sync.dma_start(wsks[:], wskT[:])
            nc.sync.dma_start(btrgs[:], btrg[:])
            nc.sync.dma_start(iota4s[:], iota4[:])
            nc.sync.dma_start(iotac[:], iota_c[:])
            nc.sync.dma_start(eidxs[:], eidx[:])
            nc.sync.dma_start(etcols[:], etcol[:])
            nc.sync.dma_start(kidxs[:], kidx[:])

            CH = 8  # 512-node tiles per xT load chunk
            SL = (EC + 3) // 4
            with tc.tile_pool(name="pa", bufs=3) as pa, \
                 tc.tile_pool(name="pax", bufs=2) as pax, \
                 tc.tile_pool(name="pe", bufs=2) as pe, \
                 tc.tile_pool(name="pe2", bufs=3) as pe2, \
                 tc.tile_pool(name="peg", bufs=3) as peg, \
                 tc.tile_pool(name="psa", bufs=2, space="PSUM") as psa, \
                 tc.tile_pool(name="psb", bufs=2, space="PSUM") as psb, \
                 tc.tile_pool(name="psc", bufs=1, space="PSUM") as psc, \
                 tc.tile_pool(name="psd", bufs=1, space="PSUM") as psd, \
                 tc.tile_pool(name="pse", bufs=2, space="PSUM") as pse:

                # ---------------- phase A ----------------
                for t0 in range(0, NT, CH):
                    t1 = min(t0 + CH, NT)
                    wdc = (t1 - t0) * 512
                    slc = slice(t0 * 512, t0 * 512 + wdc)
                    xa = pax.tile([P, CH * 512], XDT, tag="xa")
                    nc.sync.dma_start(xa[:, 0:wdc], xT[0:P, slc])
                    xb = pax.tile([P, CH * 512], XDT, tag="xb")
                    nc.sync.dma_start(xb[:, 0:wdc], xT[P:IN, slc])
                    for t in range(t0, t1):
                        o = (t - t0) * 512
                        hps = psa.tile([P, 512], F32, tag="hps")
                        nc.tensor.matmul(hps[:], lhsT=w1a[:],
                                         rhs=xa[:, o:o + 512],
                                         start=True, stop=False)
                        nc.tensor.matmul(hps[:], lhsT=w1b[:],
                                         rhs=xb[:, o:o + 512],
                                         start=False, stop=True)
                        hsb = pa.tile([P, 512], F32R, tag="hsb")
                        nc.scalar.activation(hsb[:], hps[:], AF.Relu,
                                             bias=b1s[:])
                        if t * 512 < TP:
                            w0 = t * 512
                            w1_ = min(TP, (t + 1) * 512)
                            nc.scalar.activation(hfmt[:, w0:w1_],
                                                 hps[:, 0:(w1_ - w0)], AF.Relu,
                                                 bias=b1s[:])
                        stg = pa.tile([P, 4, 256], BF, tag="stg")
                        for half in range(2):
                            p2 = psb.tile([P, 2, 256], F32, tag="p2")
                            for jj in range(2):
                                j = half * 2 + jj
                                nc.tensor.matmul(
                                    p2[:, jj, :],
                                    lhsT=hsb[:, j * P:(j + 1) * P],
                                    rhs=w2s[:], start=True, stop=True)
                            sgh = stg[:, half * 2:half * 2 + 2, :]
                            if half == 0:
                                nc.scalar.activation(sgh[:, :, 0:OC],
                                                     p2[:, :, 0:OC], AF.Copy)
                            else:
                                nc.vector.tensor_copy(sgh[:, :, 0:OC],
                                                      p2[:, :, 0:OC])
                            # s_src f32 bits + zero pad tail in one copy (w2
                            # cols 136:192 are zero -> psum cols are zero)
                            nc.vector.tensor_copy(
                                sgh[:, :, OC:256].bitcast(F32),
                                p2[:, :, OC:OC + 64])
                        r0 = t * 512
                        nc.sync.dma_start(
                            tabA[r0:r0 + 512, :].rearrange(
                                "(j p) f -> p j f", p=P), stg[:])

                # ---------------- loop 1a: per-window s_trg / skip ----------
                for w in range(W):
                    stp = psd.tile([P, OC], F32, tag="misc")
                    nc.tensor.matmul(stp[:, 0:NH],
                                     lhsT=hfmt[:, w * P:(w + 1) * P],
                                     rhs=btrgs[:], start=True, stop=True)
                    nc.vector.tensor_copy(strg[:, w * NH:(w + 1) * NH],
                                          stp[:, 0:NH])
                    skp = psd.tile([P, OC], F32, tag="misc")
                    nc.tensor.matmul(skp[:], lhsT=hfmt[:, w * P:(w + 1) * P],
                                     rhs=wsks[:], start=True, stop=True)
                    nc.vector.tensor_copy(skips[:, w], skp[:])

                # ---------------- loop 1b: s_trg -> edge slots --------------
                for w in range(W if PARTS >= 3 else 0):
                    etw = pe.tile([1, cap], BF, tag="etw")
                    nc.sync.dma_start(etw[:], etrow[0:1, w * cap:(w + 1) * cap])
                    pbcw = pe.tile([P, cap], BF, tag="pbcw")
                    nc.gpsimd.partition_broadcast(pbcw[:], etw[:])
                    Mtw = pe.tile([P, cap], BF, tag="Mtw")
                    nc.vector.tensor_scalar(Mtw[:], pbcw[:], iotac[:], None,
                                            OP.is_equal)
                    stps = psc.tile([P, EC, NH], F32, tag="stps")
                    for j in range(EC):
                        nc.tensor.matmul(
                            stps[:, j, :], lhsT=Mtw[:, j * P:(j + 1) * P],
                            rhs=strg[:, w * NH:(w + 1) * NH],
                            start=True, stop=True)
                    nc.vector.tensor_copy(st_sb[:, w], stps[:])

                # ---------------- loop 2: per-window edge pipeline ----------
                # finalize is deferred one iteration so the late ELU chain of
                # window w doesn't block window w+1's early ops in the
                # in-order ACT/DVE streams.
                def finalize(w, segp):
                    den = pe2.tile([P, NH], F32, tag="den")
                    nc.vector.tensor_scalar_add(den[:], segp[:, OC:OC + NH],
                                                EPS)
                    rec = pe2.tile([P, NH], F32, tag="rec")
                    nc.vector.reciprocal(rec[:], den[:])
                    z = pe2.tile([P, OC], F32, tag="z")
                    recb = rec[:].broadcast_to([P, NH, HD])
                    nc.vector.tensor_tensor(
                        z[:].rearrange("p (a d) -> p a d", d=HD),
                        segp[:, 0:OC].rearrange("p (a d) -> p a d", d=HD),
                        recb, OP.mult)
                    nc.vector.tensor_add(z[:], z[:], skips[:, w])
                    # elu: (max(z,0)-1) + exp(min(z,0))
                    am = pe2.tile([P, OC], F32, tag="am")
                    nc.vector.tensor_scalar(am[:], z[:], 0.0, -1.0, OP.max,
                                            OP.add)
                    bm = pe2.tile([P, OC], F32, tag="bm")
                    nc.vector.tensor_scalar(bm[:], z[:], 0.0, None, OP.min)
                    eb = pe2.tile([P, OC], F32, tag="eb")
                    nc.scalar.activation(eb[:], bm[:], AF.Exp)
                    fo = pe2.tile([P, OC], F32, tag="fo")
                    nc.vector.tensor_add(fo[:], am[:], eb[:])
                    nc.sync.dma_start(outT[w * P:(w + 1) * P, :], fo[:])

                pending = None
                for w in range(W if PARTS >= 2 else 0):
                    G = peg.tile([P, EC, 256], BF, tag="G")
                    nc.gpsimd.dma_gather(
                        G[:], tabA[:], eidxs[:, w * EC * 8:(w + 1) * EC * 8],
                        cap, cap, 256, single_packet=False)

                    if PARTS < 3:
                        continue
                    Mw = pe2.tile([P, EC * P], BF, tag="Mw")
                    for j in range(EC):
                        nc.vector.tensor_scalar(
                            Mw[:, j * P:(j + 1) * P], iota4s[:, 0:P],
                            etcols[:, w * EC + j:w * EC + j + 1], None,
                            OP.is_equal)

                    # scores = s_src(gathered) + s_trg(expanded)
                    sc = pe2.tile([P, EC, NH], F32, tag="sc")
                    gss = G[:, :, OC:OC + 2 * NH].bitcast(F32)
                    nc.vector.tensor_tensor(sc[:], st_sb[:, w], gss, OP.add)
                    # exp(leakyrelu(s)) = max(exp(s), exp(0.2 s))  (monotonic)
                    e1 = pe2.tile([P, EC, NH], BF, tag="e1")
                    nc.scalar.activation(e1[:], sc[:], AF.Exp)
                    e2 = pe2.tile([P, EC, NH], BF, tag="e2")
                    nc.scalar.activation(e2[:], sc[:], AF.Exp, scale=0.2)
                    emax = pe2.tile([P, EC, NH], BF, tag="emax")
                    nc.vector.tensor_max(emax[:], e1[:], e2[:])
                    Wv = pe2.tile([P, EC, 136], BF, tag="Wv")
                    nc.vector.tensor_copy(Wv[:, :, OC:OC + NH], emax[:])
                    # expand exp per-head on ACT (broadcast read), then a
                    # fully packed bf16 multiply on DVE (2x mode)
                    eex = pe2.tile([P, EC, OC], BF, tag="eex")
                    nc.scalar.activation(
                        eex[:].rearrange("p j (a d) -> p j a d", d=HD),
                        emax[:].broadcast_to([P, EC, NH, HD]), AF.Copy)
                    nc.vector.tensor_tensor(Wv[:, :, 0:OC], G[:, :, 0:OC],
                                            eex[:], OP.mult)

                    if PARTS < 4:
                        continue
                    segp = pse.tile([P, 136], F32, tag="segp")
                    for j in range(EC):
                        nc.tensor.matmul(segp[:], lhsT=Mw[:, j * P:(j + 1) * P],
                                         rhs=Wv[:, j, :], start=(j == 0),
                                         stop=(j == EC - 1))
                    if pending is not None:
                        finalize(*pending)
                    pending = (w, segp)
                if pending is not None and PARTS >= 4:
                    finalize(*pending)

                # final k-row gather
                if PARTS >= 5:
                    ko = peg.tile([P, TP // P, OC], F32, tag="ko")
                    nc.gpsimd.dma_gather(ko[:], outT[:], kidxs[:], TP, TP, OC,
                                         single_packet=False)
                    nc.sync.dma_start(
                        out[:].rearrange("(j p) f -> p j f", p=P), ko[:])

    nc.compile()
    _BUILD_CACHE[key] = nc
    return nc


# ----------------------------------------------------------------------------
# entry point
# ----------------------------------------------------------------------------

def kernel(x, adj0, index0, w_in, b_in, w_proj, a_src, a_trg, w_skip):
    from concourse.bass_utils import run_bass_kernel_spmd

    per_core, NPAD, EC = plan(x, adj0, index0)
    wts = make_weights(w_in, b_in, w_proj, a_src, a_trg, w_skip)
    nc = build(NPAD, EC)

    in_maps = []
    for c in range(CORES):
        m = dict(wts)
        m.update(per_core[c])
        in_maps.append(m)

    res = run_bass_kernel_spmd(nc, in_maps, core_ids=list(range(CORES)))
    outs = [r["out"][:KC] for r in res.results]
    return np.concatenate(outs, axis=0).astype(np.float32)
